# revision 22
# baseline (speedup 1.0000x reference)
"""Trainium2 Bass kernel for nn_Confidence_Score (gnn_message_passing).

Math: with S_g = sum of x over nodes of graph g and n_g = node count,
every node of graph g has identical activations:
    h1_g = relu(S_g @ W1 + b1)
    h2_g = relu((n_g * h1_g) @ W2 + b2)
    c_g  = h2_g @ Wc + bc ;  out_node = sp/(1+sp), sp = softplus(c_g)

v3 design (bf16 throughout, tolerance 2e-2):
 - pass 1 (segment sum): per 128-node chunk ONE matmul with lhsT=x_chunk
   (weights) and rhs=NR host-built 0/1 mask columns -> per-(chunk,run)
   partial sums land in PSUM s_runsT[feat, NR*c+r]. No on-chip one-hot.
   Sorted batch => runs are contiguous graph segments; NR=3 suffices.
 - run->graph reduction: transpose s_runsT (PE) and contract with a
   host 0/1 matrix R via 3 matmuls -> S^T[feat, graph]. Structure is
   batch-independent, so one SPMD program serves all 8 cores.
 - MLP entirely in transposed orientation (no PE transposes):
   h1T = relu(W1^T S^T + b1), h2T = relu(n*(W2^T h1T) + b2),
   c = h2T^T wc (PE, lands as a [G,1] column), softplus via ACT table.
 - pass 2: out = og^T @ A_T; A_T = is_eq(broadcast(batch), iota) built
   on DVE during the pass-1 DMA window; batch broadcast via GpSimd
   partition_broadcast (keeps the DMA rings free for x).
 - x as bf16 [N,128]; out returned bf16, cast on host.

Sharding: graph-aligned contiguous node ranges, balanced by node count,
one range per core (8 cores); weights replicated; no collectives.
"""

import os
import sys

for _p in ("/root/.axon_site", "/root/.axon_site/_ro/trn_rl_repo",
           "/root/.axon_site/_ro/pypackages", "/opt/trn_rl_repo"):
    if os.path.isdir(_p) and _p not in sys.path:
        sys.path.append(_p)

import numpy as np

N_CORES = 8
D = 128
H = 256
G_TOTAL = 512
G_PAD = 72        # max local graphs per core (actual ~66)
CHUNK = 128       # nodes per aggregation matmul
XB = 20           # chunks per x DMA group
NR = 3            # mask/run columns per chunk
OB = 512          # nodes per expansion matmul / A_T-gen block

# maskR bf16 tile columns (mkr [128, MKR]): masks | R-slices | ident
M_MASK = 0                      # [128, NR*n_chunks]
# (R slices and ident appended after masks; offsets depend on n_chunks)

# cpkw bf16 tile columns (weights, [128, CPW]):
W_W1 = 0            # w1 [128,256]
W_W2A = 256         # w2 rows 0-127 [128,256]
W_W2B = 512         # w2 rows 128-255 [128,256]
W_WC = 768          # wc as 2 cols
W_BCR = 770         # bc broadcast row (partition 0) [1, G_PAD]
CPW = 770 + G_PAD

# f32 const tile columns (cpkf [128, CPF]):
F_B1A = 0           # b1[0:128] col
F_B1B = 1
F_B2A = 2
F_B2B = 3
F_NC = 4            # ncol [G_PAD,1]
F_BC = 5            # bccol [G_PAD,1]
F_IO = 6            # iota column 0..127
CPF = 8

_CACHE = {}


def _build(nodes_pad):
    """Build + compile the single-core Bass program (shapes uniform across cores)."""
    from contextlib import ExitStack

    import concourse.bacc as bacc
    import concourse.mybir as mybir
    import concourse.tile as tile

    f32 = mybir.dt.float32
    bf16 = mybir.dt.bfloat16
    u8 = mybir.dt.uint8
    AF = mybir.ActivationFunctionType
    OP = mybir.AluOpType

    n_chunks = nodes_pad // CHUNK
    assert n_chunks % XB == 0
    n_groups = n_chunks // XB
    n_ob = nodes_pad // OB
    nruns = NR * n_chunks
    n_rs = -(-nruns // 128)          # K-slices for the run reduction
    MK_R = NR * n_chunks             # R slices at [MK_R, MK_R + n_rs*G_PAD)
    MK_ID = MK_R + n_rs * G_PAD      # ident [128,128]
    MKR = MK_ID + 128

    nc = bacc.Bacc("TRN2", target_bir_lowering=False, debug=False)

    xb_d = nc.dram_tensor("xb", [nodes_pad, D], bf16, kind="ExternalInput").ap()
    bta_d = nc.dram_tensor("btall", [1, nodes_pad], u8, kind="ExternalInput").ap()
    mkr_d = nc.dram_tensor("mkr", [128, MKR], bf16, kind="ExternalInput").ap()
    cpw_d = nc.dram_tensor("cpkw", [128, CPW], bf16, kind="ExternalInput").ap()
    cpf_d = nc.dram_tensor("cpkf", [128, CPF], f32, kind="ExternalInput").ap()
    cpe_d = nc.dram_tensor("cpke", [128, 2], f32, kind="ExternalInput").ap()
    nbr_d = nc.dram_tensor("nbrow", [1, G_PAD], f32, kind="ExternalInput").ap()
    out_d = nc.dram_tensor("out", [n_ob, OB], bf16, kind="ExternalOutput").ap()

    # host pre-shuffles xb so each (group, partition) segment is contiguous
    xb_groups = xb_d.rearrange("(g p j) d -> g p (j d)", p=CHUNK, j=XB)

    with tile.TileContext(nc) as tc, ExitStack() as ctx:
        const = ctx.enter_context(tc.tile_pool(name="const", bufs=1))
        store = ctx.enter_context(tc.tile_pool(name="store", bufs=1))
        ps_s = ctx.enter_context(tc.tile_pool(name="ps_s", bufs=1, space="PSUM"))

        # sync ring: masks/R/ident then x groups 0/2/4.
        # scalar ring: tiny consts, x groups 1/3, batch row + SBUF-side
        # broadcast pieces, then MLP consts.
        mkr = const.tile([128, MKR], bf16)
        nc.sync.dma_start(mkr[:], mkr_d[:])
        cpe = const.tile([128, 2], f32)
        nc.scalar.dma_start(cpe[:], cpe_d[:])
        btb = store.tile([G_PAD, nodes_pad], u8)
        io32 = cpe[0:G_PAD, 0:1]

        masks = mkr[:, 0:NR * n_chunks]
        ident = mkr[:, MK_ID:MK_ID + 128]
        cpw = const.tile([128, CPW], bf16)
        cpf = const.tile([128, CPF], f32)
        nb = const.tile([128, G_PAD], f32)

        w1_s = cpw[:, W_W1:W_W1 + H]
        w2a = cpw[:, W_W2A:W_W2A + H]
        w2b = cpw[:, W_W2B:W_W2B + H]
        wca = cpw[:, W_WC:W_WC + 1]
        wcb = cpw[:, W_WC + 1:W_WC + 2]
        b1a = cpf[:, F_B1A:F_B1A + 1]
        b1b = cpf[:, F_B1B:F_B1B + 1]
        b2a = cpf[:, F_B2A:F_B2A + 1]
        b2b = cpf[:, F_B2B:F_B2B + 1]
        bcs = cpe[0:G_PAD, 1:2]

        at_sb = store.tile([G_PAD, nodes_pad], bf16)
        sr_ps = ps_s.tile([128, nruns], f32)
        zz = const.tile([G_PAD, 32], f32)
        nc.vector.memset(zz[:], 0.0)

        # ---- pass 1: per-chunk run sums + A_T generation ----
        nodes_g = XB * CHUNK
        at_per_g = -(-n_ob // (n_groups - 2))
        ps_w = ctx.enter_context(tc.tile_pool(name="ps_w", bufs=1, space="PSUM"))
        warm = ps_w.tile([64, 64], f32)

        def warmers(k):
            for _ in range(k):
                nc.tensor.matmul(
                    warm[:], lhsT=ident[:, 0:64], rhs=ident[:, 0:64],
                    start=True, stop=True)

        with tc.tile_pool(name="xp", bufs=n_groups) as xpool:
            for g in range(n_groups):
                xt = xpool.tile([CHUNK, XB * D], bf16)
                eng = nc.sync if g % 2 == 0 else nc.scalar
                eng.dma_start(xt[:], xb_groups[g])
                if g == 1:
                    # batch broadcast pieces behind xt1 on scalar ring
                    for p in range(2):
                        nc.scalar.dma_start(
                            btb[:, p * nodes_g:(p + 1) * nodes_g],
                            bta_d[0:1, p * nodes_g:(p + 1) * nodes_g]
                            .to_broadcast((G_PAD, nodes_g)))
                if g == 3:
                    for p in range(2, n_groups):
                        nc.scalar.dma_start(
                            btb[:, p * nodes_g:(p + 1) * nodes_g],
                            bta_d[0:1, p * nodes_g:(p + 1) * nodes_g]
                            .to_broadcast((G_PAD, nodes_g)))
                    # weight/bias consts ride the scalar ring behind xt3
                    nc.scalar.dma_start(cpw[:], cpw_d[:])
                    nc.scalar.dma_start(cpf[:], cpf_d[:])
                    nc.scalar.dma_start(
                        nb[:], nbr_d[0:1, :].to_broadcast((128, G_PAD)))
                for j in range(XB):
                    c = g * XB + j
                    nc.tensor.matmul(
                        sr_ps[:, NR * c:NR * (c + 1)],
                        lhsT=xt[:, j * D:(j + 1) * D],
                        rhs=masks[:, NR * c:NR * (c + 1)],
                        start=True, stop=True,
                    )
                if g < n_groups - 1:
                    warmers(16)
                if g >= 2:
                    lo = (g - 2) * at_per_g
                    hi = n_ob if g == n_groups - 1 else min((g - 1) * at_per_g, n_ob)
                    for tb in range(lo, hi):
                        nc.vector.tensor_scalar(
                            at_sb[:, tb * OB:(tb + 1) * OB],
                            btb[:, tb * OB:(tb + 1) * OB], io32,
                            None, op0=OP.is_equal,
                        )

        # ---- run->graph reduction + per-graph MLP (transposed layout) ----
        with (
            tc.tile_pool(name="mlp", bufs=1) as mlp,
            tc.tile_pool(name="ps_m", bufs=1, space="PSUM") as ps_m,
        ):
            sr_sb = mlp.tile([128, nruns], bf16)
            nc.vector.tensor_copy(sr_sb[:], sr_ps[:])
            st_ps = ps_m.tile([128, G_PAD], f32, tag="mm")
            for k in range(n_rs):
                kk = min(128, nruns - k * 128)
                tp = ps_m.tile([128, 128], bf16, tag="tps")
                nc.tensor.transpose(
                    tp[0:kk, 0:128], sr_sb[:, k * 128:k * 128 + kk],
                    ident[:, :])
                tsb = mlp.tile([128, 128], bf16, tag=f"tr{k}")
                nc.vector.tensor_copy(tsb[0:kk, :], tp[0:kk, 0:128])
                nc.tensor.matmul(
                    st_ps[:], lhsT=tsb[0:kk, :],
                    rhs=mkr[0:kk, MK_R + k * G_PAD:MK_R + (k + 1) * G_PAD],
                    start=(k == 0), stop=(k == n_rs - 1),
                )
            st_sb = mlp.tile([128, G_PAD], bf16)
            nc.vector.tensor_copy(st_sb[:], st_ps[:])

            # h1T = relu(W1^T S^T + b1) as two [128, G] halves
            h1sb = []
            for k, bcol in ((0, b1a), (1, b1b)):
                hp = ps_m.tile([128, G_PAD], f32, tag="hh")
                nc.tensor.matmul(
                    hp[:], lhsT=w1_s[:, k * 128:(k + 1) * 128], rhs=st_sb[:],
                    start=True, stop=True)
                hs = mlp.tile([128, G_PAD], bf16, tag=f"h1s{k}")
                nc.vector.tensor_scalar(
                    hs[:], hp[:], bcol, 0.0, op0=OP.add, op1=OP.max)
                h1sb.append(hs)

            # h2T = relu(n * (W2^T h1T) + b2)
            h2sb = []
            for k, bcol in ((0, b2a), (1, b2b)):
                hp = ps_m.tile([128, G_PAD], f32, tag="hh")
                for m in range(2):
                    nc.tensor.matmul(
                        hp[:],
                        lhsT=(w2a if m == 0 else w2b)[:, k * 128:(k + 1) * 128],
                        rhs=h1sb[m][:], start=(m == 0), stop=(m == 1))
                hn = mlp.tile([128, G_PAD], f32, tag=f"h2n{k}")
                nc.vector.tensor_tensor(hn[:], hp[:], nb[:], op=OP.mult)
                hs = mlp.tile([128, G_PAD], bf16, tag=f"h2s{k}")
                nc.vector.tensor_scalar(
                    hs[:], hn[:], bcol, 0.0, op0=OP.add, op1=OP.max)
                h2sb.append(hs)

            # c = h2T^T wc + bc -> [G_PAD, 1] column (bc via a K=1 matmul
            # against ident[0,0] == 1)
            c_ps = ps_m.tile([G_PAD, 1], f32, tag="cc")
            for m in range(2):
                nc.tensor.matmul(
                    c_ps[:], lhsT=h2sb[m][:], rhs=(wca if m == 0 else wcb),
                    start=(m == 0), stop=False)
            nc.tensor.matmul(
                c_ps[:], lhsT=cpw[0:1, W_BCR:W_BCR + G_PAD],
                rhs=ident[0:1, 0:1], start=False, stop=True)

            # og = 1 - 1/(1+sp); sp = relu(c) + ln1p(exp(-|c|)).
            # ln1p(u) ~ u*((q3 u + q2) u^2 + (q1 u + q0)), max out err ~1e-4.
            negc = mlp.tile([G_PAD, 1], f32)
            nc.vector.tensor_scalar_mul(negc[:], c_ps[:], -1.0)
            nab = mlp.tile([G_PAD, 1], f32)
            nc.vector.tensor_tensor(nab[:], c_ps[:], negc[:], op=OP.min)
            u = mlp.tile([G_PAD, 1], f32)
            nc.scalar.activation(u[:], nab[:], AF.Exp)
            warmers(10)
            rl = mlp.tile([G_PAD, 1], f32)
            nc.vector.tensor_scalar_max(rl[:], c_ps[:], 0.0)
            qa = mlp.tile([G_PAD, 1], f32)
            nc.vector.tensor_scalar(
                qa[:], u[:], -0.07473615, 0.25462221, op0=OP.mult, op1=OP.add)
            qb = mlp.tile([G_PAD, 1], f32)
            nc.vector.tensor_scalar(
                qb[:], u[:], -0.48664306, 0.99962038, op0=OP.mult, op1=OP.add)
            u2 = mlp.tile([G_PAD, 1], f32)
            nc.vector.tensor_tensor(u2[:], u[:], u[:], op=OP.mult)
            nc.vector.tensor_tensor(qa[:], qa[:], u2[:], op=OP.mult)
            nc.vector.tensor_tensor(qa[:], qa[:], qb[:], op=OP.add)
            sp = mlp.tile([G_PAD, 1], f32)
            nc.vector.tensor_tensor(sp[:], qa[:], u[:], op=OP.mult)
            nc.vector.tensor_tensor(sp[:], sp[:], rl[:], op=OP.add)
            t1 = mlp.tile([G_PAD, 1], f32)
            nc.vector.tensor_scalar_add(t1[:], sp[:], 1.0)
            rcp = mlp.tile([G_PAD, 1], f32)
            nc.vector.reciprocal(rcp[:], t1[:])
            ogr = const.tile([G_PAD, 32], bf16)
            nc.vector.tensor_scalar(
                ogr[:], zz[:], rcp[:], 1.0, op0=OP.subtract, op1=OP.add)

        # ---- pass 2: out = og.T @ A_T, 512 nodes per matmul ----
        # block b (= r*NQ + q) -> bank-tile q, partition band 32*r
        NQ = (n_ob + 2) // 3
        es2 = store.tile([96, NQ * OB], bf16)
        with tc.tile_pool(name="ps_e", bufs=4, space="PSUM") as ps_e:
            for q in range(NQ):
                e_ps = ps_e.tile([96, OB], f32)
                for r in range(3):
                    b = r * NQ + q
                    if b >= n_ob:
                        continue
                    nc.tensor.matmul(
                        e_ps[32 * r:32 * r + 32, :], lhsT=ogr[:],
                        rhs=at_sb[:, b * OB:(b + 1) * OB],
                        start=True, stop=True,
                    )
                hb = OB // 2
                nc.vector.tensor_copy(
                    es2[:, q * OB:q * OB + hb], e_ps[:, 0:hb])
                nc.scalar.copy(
                    es2[:, q * OB + hb:(q + 1) * OB], e_ps[:, hb:OB])
            # out DMAs in (band, half) pieces so they start early
            h0 = NQ // 2
            di = 0
            for r in range(3):
                nbq = min(NQ, n_ob - r * NQ)
                if nbq <= 0:
                    continue
                for qlo, qhi in ((0, min(h0, nbq)), (min(h0, nbq), nbq)):
                    if qhi <= qlo:
                        continue
                    e = nc.sync if di % 2 == 0 else nc.scalar
                    di += 1
                    e.dma_start(
                        out_d[r * NQ + qlo:r * NQ + qhi, :].rearrange(
                            "a i -> (a i)"),
                        es2[32 * r:32 * r + 1, qlo * OB:qhi * OB],
                    )

    nc.compile()
    return nc


def _shard(batch):
    """Graph-aligned split of nodes across cores, balanced by node count."""
    n = batch.shape[0]
    counts = np.bincount(batch, minlength=G_TOTAL).astype(np.int64)
    bounds = np.concatenate([[0], np.cumsum(counts)])
    gsplit = [0]
    for k in range(1, N_CORES):
        t = k * n // N_CORES
        g = int(np.searchsorted(bounds, t))
        if g > 0 and abs(int(bounds[g - 1]) - t) < abs(int(bounds[g]) - t):
            g -= 1
        g = min(max(g, gsplit[-1]), G_TOTAL)
        gsplit.append(g)
    gsplit.append(G_TOTAL)
    return counts, bounds, gsplit


def kernel(**inputs):
    import ml_dtypes
    from concourse.bass_utils import run_bass_kernel_spmd

    bf16 = ml_dtypes.bfloat16
    x = np.ascontiguousarray(np.asarray(inputs["x"], dtype=np.float32))
    batch = np.asarray(inputs["batch"]).astype(np.int64)
    W1 = np.asarray(inputs["W1"], dtype=np.float32)
    b1 = np.asarray(inputs["b1"], dtype=np.float32)
    W2 = np.asarray(inputs["W2"], dtype=np.float32)
    b2 = np.asarray(inputs["b2"], dtype=np.float32)
    Wc = np.asarray(inputs["Wc"], dtype=np.float32).reshape(H, 1)
    bc = np.asarray(inputs["bc"], dtype=np.float32).reshape(1)

    n = batch.shape[0]
    counts, bounds, gsplit = _shard(batch)
    node_cnt = [int(bounds[gsplit[k + 1]] - bounds[gsplit[k]]) for k in range(N_CORES)]
    pad_unit = np.lcm(CHUNK * XB, OB)  # DMA-group and expansion-block aligned
    nodes_pad = int(-(-max(node_cnt) // pad_unit) * pad_unit)
    assert nodes_pad % OB == 0
    assert max(gsplit[k + 1] - gsplit[k] for k in range(N_CORES)) <= G_PAD

    n_chunks = nodes_pad // CHUNK
    nruns = NR * n_chunks
    n_rs = -(-nruns // 128)
    MK_R = nruns
    MK_ID = MK_R + n_rs * G_PAD
    MKR = MK_ID + 128

    key = nodes_pad
    if key not in _CACHE:
        _CACHE[key] = _build(nodes_pad)
    nc = _CACHE[key]

    cpw = np.zeros((128, CPW), dtype=bf16)
    cpw[:, W_W1:W_W1 + H] = W1.astype(bf16)
    cpw[:, W_W2A:W_W2A + H] = W2[0:128].astype(bf16)
    cpw[:, W_W2B:W_W2B + H] = W2[128:256].astype(bf16)
    cpw[:, W_WC] = Wc[0:128, 0].astype(bf16)
    cpw[:, W_WC + 1] = Wc[128:256, 0].astype(bf16)
    cpw[0, W_BCR:W_BCR + G_PAD] = bc[0]

    cpf = np.zeros((128, CPF), dtype=np.float32)
    cpf[:, F_B1A] = b1[0:128]
    cpf[:, F_B1B] = b1[128:256]
    cpf[:, F_B2A] = b2[0:128]
    cpf[:, F_B2B] = b2[128:256]
    cpf[:, F_BC] = bc[0]
    cpf[:, F_IO] = np.arange(128, dtype=np.float32)
    cpe = np.zeros((128, 2), dtype=np.float32)
    cpe[:, 0] = np.arange(128, dtype=np.float32)
    cpe[:, 1] = bc[0]

    n_groups = nodes_pad // (CHUNK * XB)
    in_maps = []
    for k in range(N_CORES):
        gs, ge = gsplit[k], gsplit[k + 1]
        ns, ne = int(bounds[gs]), int(bounds[ge])
        cnt = ne - ns
        ng = ge - gs
        bt = np.full(nodes_pad, G_PAD - 1, dtype=np.int64)
        bt[:cnt] = batch[ns:ne] - gs
        xbp = np.zeros((nodes_pad, D), dtype=bf16)
        xbp[:cnt] = x[ns:ne].astype(bf16)
        # shuffle to (group, partition, chunk-in-group, row) DMA order
        xbp = np.ascontiguousarray(
            xbp.reshape(n_groups, XB, CHUNK, D).transpose(0, 2, 1, 3)
        ).reshape(nodes_pad, D)

        # masks + run->graph matrix R (runs are contiguous graph segments
        # inside each chunk; batch is sorted)
        mkr = np.zeros((128, MKR), dtype=bf16)
        Rm = np.zeros((n_rs * 128, G_PAD), dtype=np.float32)
        btc = bt.reshape(n_chunks, CHUNK)
        for c in range(n_chunks):
            lo = c * CHUNK
            if lo >= cnt:
                break  # pure-pad chunk: masks/R stay zero
            valid = min(CHUNK, cnt - lo)
            row = btc[c][:valid]
            glo, ghi = int(row[0]), int(row[valid - 1])
            nr = ghi - glo + 1
            assert nr <= NR, f"chunk {c} spans {nr} graphs > NR"
            for r in range(nr):
                mkr[:valid, NR * c + r] = (row == glo + r).astype(np.float32)
                Rm[NR * c + r, glo + r] = 1.0
        mkr[:, MK_R:MK_ID] = np.ascontiguousarray(
            Rm.reshape(n_rs, 128, G_PAD).transpose(1, 0, 2).reshape(
                128, n_rs * G_PAD)).astype(bf16)
        mkr[:, MK_ID:MK_ID + 128] = np.eye(128, dtype=np.float32)

        cpfk = cpf.copy()
        cpfk[0:ng, F_NC] = counts[gs:ge].astype(np.float32)
        nbrow = np.zeros((1, G_PAD), dtype=np.float32)
        nbrow[0, 0:ng] = counts[gs:ge].astype(np.float32)

        in_maps.append({
            "xb": xbp,
            "btall": np.ascontiguousarray(bt.astype(np.uint8).reshape(1, nodes_pad)),
            "mkr": mkr,
            "cpkw": cpw,
            "cpkf": cpfk,
            "cpke": cpe,
            "nbrow": nbrow,
        })

    res = run_bass_kernel_spmd(nc, in_maps, core_ids=list(range(N_CORES)))
    outs = []
    for k in range(N_CORES):
        o = res.results[k]["out"].reshape(-1)
        outs.append(o[: node_cnt[k]])
    return np.concatenate(outs).reshape(n, 1).astype(np.float32)
